# revision 1
# baseline (speedup 1.0000x reference)
"""DNeRF-TensoRF sampler kernel for Trainium2 (8 NeuronCores, data-parallel over points).

Strategy:
  - Host pre-packs the 9 feature planes (3 feats x 3 components) into 3 gather
    tables PT_j of shape (NUM_FRAMES*RESO, 384) fp32 where row r = t*RESO+x holds
    [A | D]: A[f*64+c] = feat_f[j,c,t,x], D = A(next x) - A  (difference table, so
    the device lerp is A + wx*D).  The y (time) coordinate is an exact integer
    frame index, so the bilinear reduces to a 1-D lerp along x at row t.
  - x is sharded over 8 cores along the point axis.  Each core processes
    PC = P/8 points: computes int16 row indices on 16 partitions (the layout
    dma_gather needs), gathers 1536B rows per (point, component) with SWDGE
    dma_gather, lerps, multiplies the 3 component samples, reduces over 64
    channels, and evaluates the sin/cos positional encoding on the ACT engine.
  - Outputs are written in an interleaved device layout and un-permuted on host.
"""
import sys

sys.path.insert(0, "/opt/trn_rl_repo")

from contextlib import ExitStack

import numpy as np

import concourse.bacc as bacc
import concourse.bass as bass
import concourse.mybir as mybir
import concourse.tile as tile
from concourse.bass_utils import run_bass_kernel_spmd

NUM_FRAMES = 100
RESO = 256
CHAN = 64
FREQ = 10
P = 524288
NCORES = 8
PC = P // NCORES            # 65536 points per core
NROWS = NUM_FRAMES * RESO   # 25600 table rows
ES = 2 * 3 * CHAN           # 384 floats per gather row ([A|D], 3 feats x 64 ch)
NB = 1024                   # points per gather batch
K_RED = 64 * np.pi          # even multiple of 2*pi used to make mod args positive

TWO_PI = float(2 * np.pi)
INV_TWO_PI = float(1.0 / (2 * np.pi))
# largest fp32 strictly below float64 pi (Sin activation domain is [-pi, pi])
PI_LO = float(np.nextafter(np.float32(np.pi), np.float32(0.0)))


def build_program(pc=PC, nb=NB, hw_round=True, reps=1, pool_prod=False):
    """hw_round: HW casts fp32->int with round-to-nearest; CoreSim truncates.
    floor(v) is computed as cast(v + CAST_OFF) with CAST_OFF=-0.5 on HW, 0 in
    sim; round(v) as cast(v + RED_OFF) with RED_OFF=0 on HW, +0.5 in sim.
    reps: repeat the main loop (timing amplification).  pool_prod: run the
    first product multiply on GPSIMD to offload the vector engine."""
    cast_off = -0.5 if hw_round else 0.0
    red_off = 0.0 if hw_round else 0.5
    m = nb // 128           # free slots per batch
    lw = pc // 16           # idx-layout free size
    mc = pc // 128          # interleaved-layout free size
    nbatch = pc // nb
    f32 = mybir.dt.float32
    i16 = mybir.dt.int16
    i32 = mybir.dt.int32
    A = mybir.AluOpType

    nc = bacc.Bacc("TRN2", target_bir_lowering=False, debug=False)

    xw = nc.dram_tensor("xw", [16, 4 * lw], f32, kind="ExternalInput")
    xc = nc.dram_tensor("xc", [128, mc * 4], f32, kind="ExternalInput")
    pts = [
        nc.dram_tensor(f"pt{j}", [NROWS, ES], f32, kind="ExternalInput")
        for j in range(3)
    ]
    fr = nc.dram_tensor("fr", [128, 30], f32, kind="ExternalInput")
    out = nc.dram_tensor("out", [128, mc * 63], f32, kind="ExternalOutput")

    with tile.TileContext(nc) as tc, ExitStack() as ctx:
        cpool = ctx.enter_context(tc.tile_pool(name="const", bufs=1))
        frt = cpool.tile([128, 30], f32)
        nc.sync.dma_start(frt[:], fr.ap()[:])
        xct = cpool.tile([128, mc * 4], f32)
        nc.sync.dma_start(xct[:], xc.ap()[:])
        wxt = cpool.tile([128, 3 * mc], f32)
        idx128 = cpool.tile([128, 3 * lw], i16)

        # ---- setup: per-point row indices on 16 partitions, weights on 128 ----
        with tc.tile_pool(name="setup", bufs=1) as spool:
            xwt = spool.tile([16, 4 * lw], f32)
            nc.sync.dma_start(xwt[:], xw.ap()[:])
            xw_t = xwt[:, 3 * lw:4 * lw]
            for j in range(3):
                xw_j = xwt[:, j * lw:(j + 1) * lw]
                ixw = spool.tile([16, lw], f32, tag="ixw")
                nc.vector.tensor_scalar(ixw[:], xw_j, 255.0, cast_off,
                                        A.mult, A.add)
                nc.vector.tensor_scalar(ixw[:], ixw[:], 0.0, None, A.max)
                ixi = spool.tile([16, lw], i32, tag="ixi")
                nc.vector.tensor_copy(ixi[:], ixw[:])   # == floor(255*x)
                x0w = spool.tile([16, lw], f32, tag="x0w")
                nc.vector.tensor_copy(x0w[:], ixi[:])
                rw = spool.tile([16, lw], f32, tag="rw")
                nc.vector.scalar_tensor_tensor(
                    rw[:], xw_t, 256.0, x0w[:], A.mult, A.add)
                idx16 = spool.tile([16, lw], i16, tag="idx16")
                nc.vector.tensor_copy(idx16[:], rw[:])
                for k in range(8):
                    nc.sync.dma_start(
                        idx128[16 * k:16 * (k + 1), j * lw:(j + 1) * lw], idx16[:])
                # interleaved-layout lerp weights: wx1 = ix - floor(ix)
                xj = xct[:].rearrange("p (q f) -> p q f", f=4)[:, :, j]
                ixc = spool.tile([128, mc], f32, tag="ixc")
                nc.vector.tensor_scalar(ixc[:], xj, 255.0, None, A.mult)
                icm = spool.tile([128, mc], f32, tag="icm")
                nc.vector.tensor_scalar(icm[:], xj, 255.0, cast_off,
                                        A.mult, A.add)
                nc.vector.tensor_scalar(icm[:], icm[:], 0.0, None, A.max)
                ici = spool.tile([128, mc], i32, tag="ici")
                nc.vector.tensor_copy(ici[:], icm[:])
                icf = spool.tile([128, mc], f32, tag="icf")
                nc.vector.tensor_copy(icf[:], ici[:])
                nc.vector.tensor_tensor(
                    wxt[:, j * mc:(j + 1) * mc], ixc[:], icf[:], A.subtract)

        xyz = xct[:].rearrange("p (q f) -> p q f", f=4)[:, :, 0:3]  # (128, mc, 3)

        gpool = ctx.enter_context(tc.tile_pool(name="g", bufs=2))
        tpool = ctx.enter_context(tc.tile_pool(name="t", bufs=2))
        opool = ctx.enter_context(tc.tile_pool(name="o", bufs=2))

        out_v = out.ap().rearrange("p (q k) -> p q k", k=63)

        for b in [bb for _ in range(reps) for bb in range(nbatch)]:
            gs = []
            for j in range(3):
                g = gpool.tile([128, m, ES], f32, tag=f"g{j}")
                nc.gpsimd.dma_gather(
                    g[:], pts[j].ap()[:],
                    idx128[:, j * lw + b * (nb // 16): j * lw + (b + 1) * (nb // 16)],
                    nb, nb, ES)
                gs.append(g)
            ss = []
            for j in range(3):
                wb = (wxt[:, j * mc + b * m: j * mc + (b + 1) * m]
                      .unsqueeze(2).to_broadcast([128, m, 192]))
                mt = tpool.tile([128, m, 192], f32, tag="mt", bufs=3)
                nc.vector.tensor_tensor(mt[:], gs[j][:, :, 192:384], wb, A.mult)
                st = tpool.tile([128, m, 192], f32, tag=f"s{j}")
                nc.vector.tensor_tensor(st[:], mt[:], gs[j][:, :, 0:192], A.add)
                ss.append(st)
            p01 = tpool.tile([128, m, 192], f32, tag="p01")
            prod_eng = nc.gpsimd if pool_prod else nc.vector
            prod_eng.tensor_tensor(p01[:], ss[0][:], ss[1][:], A.mult)
            pr = tpool.tile([128, m, 192], f32, tag="pr")
            nc.vector.tensor_tensor(pr[:], p01[:], ss[2][:], A.mult)
            delta = tpool.tile([128, m, 3], f32, tag="delta")
            nc.vector.tensor_reduce(
                delta[:], pr[:].rearrange("p q (f c) -> p q f c", c=CHAN),
                mybir.AxisListType.X, A.add)
            pxyz = tpool.tile([128, m, 3], f32, tag="pxyz")
            nc.vector.tensor_tensor(
                pxyz[:], delta[:], xyz[:, b * m:(b + 1) * m, :], A.add)

            ot = opool.tile([128, m, 63], f32, tag="ot")
            nc.scalar.activation(
                ot[:, :, 0:3], pxyz[:], mybir.ActivationFunctionType.Copy)

            ang = tpool.tile([128, m, 30], f32, tag="ang")
            nc.vector.tensor_tensor(
                ang[:].rearrange("p q (k j) -> p q k j", j=3),
                pxyz[:].unsqueeze(2).to_broadcast([128, m, FREQ, 3]),
                frt[:].rearrange("p (k j) -> p k j", j=3)
                      .unsqueeze(1).to_broadcast([128, m, FREQ, 3]),
                A.mult)
            # range reduction: w = angK - 2*pi*round(angK / 2*pi)  in [-pi, pi]
            sc_out = ot[:, :, 3:63].rearrange("p q (k s j) -> p q k s j", s=2, j=3)
            angK = tpool.tile([128, m, 30], f32, tag="angK")
            nc.vector.tensor_scalar(angK[:], ang[:], float(K_RED), None, A.add)
            for s, phase in ((0, 0.0), (1, float(np.pi / 2))):
                az = tpool.tile([128, m, 30], f32, tag="az")
                if phase:
                    nc.vector.tensor_scalar(az[:], angK[:], phase, None, A.add)
                    src = az
                else:
                    src = angK
                z = tpool.tile([128, m, 30], f32, tag="z")
                nc.vector.tensor_scalar(
                    z[:], src[:], INV_TWO_PI, red_off, A.mult, A.add)
                zi = tpool.tile([128, m, 30], i32, tag="zi")
                nc.vector.tensor_copy(zi[:], z[:])
                zf = tpool.tile([128, m, 30], f32, tag="zf")
                nc.vector.tensor_copy(zf[:], zi[:])
                wred = tpool.tile([128, m, 30], f32, tag="wred")
                nc.vector.scalar_tensor_tensor(
                    wred[:], zf[:], -TWO_PI, src[:], A.mult, A.add)
                wcl = tpool.tile([128, m, 30], f32, tag="wcl")
                nc.vector.tensor_scalar(wcl[:], wred[:], -PI_LO, None, A.max)
                nc.vector.tensor_scalar(wcl[:], wcl[:], PI_LO, None, A.min)
                nc.scalar.activation(
                    sc_out[:, :, :, s, :],
                    wcl[:].rearrange("p q (k j) -> p q k j", j=3),
                    mybir.ActivationFunctionType.Sin)

            nc.sync.dma_start(out_v[:, b * m:(b + 1) * m, :], ot[:])

    nc.compile()
    return nc


def pack_tables(feat0, feat1, feat2):
    """Build the 3 per-component gather tables (NROWS, 384) fp32 [A | D]."""
    pts = []
    for j in range(3):
        planes = np.stack([feat0[j], feat1[j], feat2[j]], axis=0)  # (3,64,100,256)
        a = np.ascontiguousarray(
            planes.transpose(2, 3, 0, 1).reshape(NROWS, 3 * CHAN)).astype(np.float32)
        d = np.zeros_like(a)
        d[:-1] = a[1:] - a[:-1]
        d[RESO - 1::RESO] = 0.0  # x=255 rows never used as base; avoid cross-frame
        pts.append(np.concatenate([a, d], axis=1))
    return pts


def pack_x(x_shard):
    """x_shard (PC,4) -> (xw (16,4*LW), xc (128,MC*4))."""
    pc = x_shard.shape[0]
    lw, mc = pc // 16, pc // 128
    xw = np.concatenate(
        [np.ascontiguousarray(x_shard[:, j].reshape(lw, 16).T) for j in range(4)],
        axis=1).astype(np.float32)
    xc = np.ascontiguousarray(
        x_shard.reshape(mc, 128, 4).transpose(1, 0, 2).reshape(128, mc * 4)
    ).astype(np.float32)
    return xw, xc


_NC_CACHE = {}


def kernel(x, feat0, feat1, feat2):
    x = np.asarray(x, dtype=np.float32)
    feat0 = np.asarray(feat0, dtype=np.float32)
    feat1 = np.asarray(feat1, dtype=np.float32)
    feat2 = np.asarray(feat2, dtype=np.float32)

    if "nc" not in _NC_CACHE:
        _NC_CACHE["nc"] = build_program()
    nc = _NC_CACHE["nc"]

    pts = pack_tables(feat0, feat1, feat2)
    fr = np.tile(np.repeat(2.0 ** np.arange(FREQ), 3).astype(np.float32)[None, :],
                 (128, 1))

    in_maps = []
    for k in range(NCORES):
        xw, xc = pack_x(x[k * PC:(k + 1) * PC])
        in_maps.append({
            "xw": xw, "xc": xc,
            "pt0": pts[0], "pt1": pts[1], "pt2": pts[2],
            "fr": fr,
        })

    res = run_bass_kernel_spmd(nc, in_maps, core_ids=list(range(NCORES)))
    outs = []
    for k in range(NCORES):
        o = res.results[k]["out"].reshape(128, PC // 128, 63)
        outs.append(o.transpose(1, 0, 2).reshape(PC, 63))
    return np.concatenate(outs, axis=0)



# revision 4
# speedup vs baseline: 2.7573x; 2.7573x over previous
"""DNeRF-TensoRF sampler kernel for Trainium2 (8 NeuronCores).

The reference bilinearly samples 9 feature planes (3 feats x 3 xyz-components)
at (x_j, t) per point; t is an exact integer frame, so sampling reduces to a
1-D lerp along x within frame t's 256-column slab.

Strategy (no per-point DMA gathers -- SWDGE descriptor generation is ~8.5ns
per descriptor and would cost ~1.7ms/core for 196k row fetches):
  - Host sorts points by frame and packs them into frame-aligned chunks of
    NP=512 points (138 chunks/core incl. dummy padding).
  - For each chunk the host ships (a) the frame's 3 slabs (256 rows x 192
    channels, bf16) and (b) three dense "hat" matrices M_j (256 x NP, bf16)
    holding the two lerp taps (1-wx, wx) per column.
  - The device computes s_j = Slab_j^T @ M_j on the tensor engine (the whole
    gather+lerp as matmuls), multiplies the three samples on DVE, and reduces
    the 64-channel blocks with a second matmul whose stationary operand is the
    product tile itself -- which lands the per-point deltas point-major in
    PSUM, ready for the positional encoding (angle-doubling recursion with
    ADD_RANGE_WRAP + ACT Sin).
  - Outputs (fp16) are unpermuted/cast on host.
"""
import sys

sys.path.insert(0, "/opt/trn_rl_repo")

from contextlib import ExitStack

import numpy as np
import ml_dtypes

import concourse.bacc as bacc
import concourse.bass as bass
import concourse.mybir as mybir
import concourse.tile as tile
from concourse.bass_utils import run_bass_kernel_spmd
from concourse.dve_ops import ADD_RANGE_WRAP

NUM_FRAMES = 100
RESO = 256
CHAN = 64
FREQ = 10
P = 524288
NCORES = 8

NP = 512                    # points per chunk (one frame per chunk)
NCH = 138                   # chunks per core
GCH = NCORES * NCH          # 1104 global chunks
STACK = 23                  # chunks per output super (138 = 6*23)
NSUP = NCH // STACK         # 6 supers
BLK = NP // 128             # 4 x 128-point blocks per chunk
SM = STACK * BLK            # 92 points per partition per super

bf16 = ml_dtypes.bfloat16
TWO_PI = float(2 * np.pi)
PI = float(np.pi)
HALF_PI = float(np.pi / 2)


def build_program():
    f32 = mybir.dt.float32
    f16 = mybir.dt.float16
    b16 = mybir.dt.bfloat16
    A = mybir.AluOpType

    nc = bacc.Bacc("TRN2", target_bir_lowering=False, debug=False)

    m_in = nc.dram_tensor("m_in", [128, NCH * 6 * NP], b16, kind="ExternalInput")
    sl_in = nc.dram_tensor("sl_in", [128, NCH * 6 * 192], b16, kind="ExternalInput")
    xyz_in = nc.dram_tensor("xyz_in", [128, NSUP * SM * 3], f32, kind="ExternalInput")
    cst_in = nc.dram_tensor("cst_in", [128, 6], f32, kind="ExternalInput")
    out = nc.dram_tensor("out", [128, NSUP * SM * 63], f16, kind="ExternalOutput")

    m_v = m_in.ap().rearrange("p (c a n) -> p c a n", a=6, n=NP)
    sl_v = sl_in.ap().rearrange("p (c a n) -> p c a n", a=6, n=192)
    out_v = out.ap().rearrange("p (s k) -> p s k", k=SM * 63)

    with tile.TileContext(nc) as tc, ExitStack() as ctx:
        cpool = ctx.enter_context(tc.tile_pool(name="const", bufs=1))
        cst = cpool.tile([128, 6], f32)
        nc.sync.dma_start(cst[:], cst_in.ap()[:])
        selw = cpool.tile([128, 6], b16)
        nc.vector.tensor_copy(selw[:], cst[:])            # cast f32 -> bf16
        xyzt = cpool.tile([128, NSUP * SM * 3], f32)
        nc.sync.dma_start(xyzt[:], xyz_in.ap()[:])
        xyz_t = xyzt[:].rearrange("p (s m d) -> p s m d", m=SM, d=3)

        mpool = ctx.enter_context(tc.tile_pool(name="m", bufs=2))
        slpool = ctx.enter_context(tc.tile_pool(name="sl", bufs=2))
        sbpool = ctx.enter_context(tc.tile_pool(name="sb", bufs=2))
        xpool = ctx.enter_context(tc.tile_pool(name="x", bufs=2))
        opool = ctx.enter_context(tc.tile_pool(name="o", bufs=2))
        pp = ctx.enter_context(tc.psum_pool(name="ps", bufs=2))

        for s in range(NSUP):
            dpm = pp.tile([128, STACK, BLK, 3], f32, tag="dpm")
            for cc in range(STACK):
                c = s * STACK + cc
                mt = mpool.tile([128, 6, NP], b16, tag="mt")
                nc.sync.dma_start(mt[:], m_v[:, c])
                sl = slpool.tile([128, 6, 192], b16, tag="sl")
                nc.sync.dma_start(sl[:], sl_v[:, c])

                sps = []
                for j in range(3):
                    sA = pp.tile([128, NP], f32, tag="sA")
                    sB = pp.tile([64, NP], f32, tag="sB")
                    nc.tensor.matmul(sA[:], sl[:, 2 * j, 0:128], mt[:, 2 * j, :],
                                     start=True, stop=False)
                    nc.tensor.matmul(sA[:], sl[:, 2 * j + 1, 0:128],
                                     mt[:, 2 * j + 1, :], start=False, stop=True)
                    nc.tensor.matmul(sB[:], sl[:, 2 * j, 128:192],
                                     mt[:, 2 * j, :], start=True, stop=False)
                    nc.tensor.matmul(sB[:], sl[:, 2 * j + 1, 128:192],
                                     mt[:, 2 * j + 1, :], start=False, stop=True)
                    sps.append((sA, sB))

                s1A = sbpool.tile([128, NP], b16, tag="s1A")
                nc.scalar.copy(s1A[:], sps[1][0][:])
                s1B = sbpool.tile([64, NP], b16, tag="s1B")
                nc.scalar.copy(s1B[:], sps[1][1][:])

                p01A = sbpool.tile([128, NP], b16, tag="p01A")
                nc.vector.tensor_tensor(p01A[:], sps[0][0][:], s1A[:], A.mult)
                p01B = sbpool.tile([64, NP], b16, tag="p01B")
                nc.vector.tensor_tensor(p01B[:], sps[0][1][:], s1B[:], A.mult)

                prA = sbpool.tile([128, NP], b16, tag="prA")
                nc.vector.tensor_tensor(prA[:], p01A[:], sps[2][0][:], A.mult)
                prB = sbpool.tile([64, NP], b16, tag="prB")
                nc.vector.tensor_tensor(prB[:], p01B[:], sps[2][1][:], A.mult)

                # channel reduce with point-major output:
                # dpm[i, cc, b, f] = sum_c pr[c, b*128+i] * selw[c, f]
                for b in range(BLK):
                    nc.tensor.matmul(dpm[:, cc, b, :],
                                     prA[:, b * 128:(b + 1) * 128],
                                     selw[:, 0:3], start=True, stop=False)
                    nc.tensor.matmul(dpm[:, cc, b, :],
                                     prB[:, b * 128:(b + 1) * 128],
                                     selw[0:64, 3:6], start=False, stop=True)

            px = xpool.tile([128, SM, 3], f32, tag="px")
            nc.vector.tensor_tensor(
                px[:], dpm[:].rearrange("p st b d -> p (st b) d"), xyz_t[:, s],
                A.add)

            ot = opool.tile([128, SM, 63], f16, tag="ot")
            nc.scalar.copy(ot[:, :, 0:3], px[:])

            th = px
            for k in range(FREQ):
                nc.scalar.activation(
                    ot[:, :, 3 + 6 * k:6 + 6 * k], th[:],
                    mybir.ActivationFunctionType.Sin)
                cw = xpool.tile([128, SM, 3], f32, tag="cw")
                nc.vector._custom_dve(ADD_RANGE_WRAP, out=cw[:], in0=th[:],
                                      s0=HALF_PI, s1=PI, imm2=TWO_PI)
                nc.scalar.activation(
                    ot[:, :, 6 + 6 * k:9 + 6 * k], cw[:],
                    mybir.ActivationFunctionType.Sin)
                if k < FREQ - 1:
                    th2 = xpool.tile([128, SM, 3], f32, tag="th2")
                    nc.vector.tensor_scalar(th2[:], th[:], 2.0, None, A.mult)
                    thn = xpool.tile([128, SM, 3], f32, tag="thn")
                    nc.vector._custom_dve(ADD_RANGE_WRAP, out=thn[:], in0=th2[:],
                                          s0=0.0, s1=PI, imm2=TWO_PI)
                    th = thn

            nc.sync.dma_start(out_v[:, s], ot[:].rearrange("p m k -> p (m k)"))

    nc.compile()
    return nc


def pack_inputs(x, feat0, feat1, feat2):
    """Sort by frame, chunk, and build per-core input maps + the scatter map
    for unpermuting device outputs."""
    x = np.asarray(x, np.float32)
    t_all = x[:, 3].astype(np.int32)

    order = np.argsort(t_all, kind="stable")
    t_sorted = t_all[order]
    counts = np.bincount(t_sorted, minlength=NUM_FRAMES)
    chunks_frame = []
    chunks_pts = []
    pos = 0
    for f in range(NUM_FRAMES):
        n = counts[f]
        fpts = order[pos:pos + n]
        pos += n
        for a in range(0, n, NP):
            seg = fpts[a:a + NP]
            if len(seg) < NP:
                seg = np.concatenate(
                    [seg, np.full(NP - len(seg), -1, np.int64)])
            chunks_frame.append(f)
            chunks_pts.append(seg)
    assert len(chunks_frame) <= GCH, f"{len(chunks_frame)} chunks > {GCH}"
    while len(chunks_frame) < GCH:
        chunks_frame.append(0)
        chunks_pts.append(np.full(NP, -1, np.int64))
    chunks_frame = np.asarray(chunks_frame, np.int32)
    chunks_pts = np.stack(chunks_pts)                    # (GCH, NP)

    # per-frame slabs: slab[f][j] = (256 rows, 192 ch), f-major channels
    slabs = np.empty((NUM_FRAMES, 3, 256, 192), np.float32)
    for j in range(3):
        planes = np.stack([np.asarray(feat0, np.float32)[j],
                           np.asarray(feat1, np.float32)[j],
                           np.asarray(feat2, np.float32)[j]], axis=0)
        slabs[:, j] = planes.transpose(2, 3, 0, 1).reshape(
            NUM_FRAMES, 256, 192)
    slabs16 = slabs.astype(bf16)

    valid = chunks_pts >= 0                              # (GCH, NP)
    safe_pts = np.where(valid, chunks_pts, 0)
    xs = x[safe_pts]                                     # (GCH, NP, 4)
    in_maps = []
    scatter = []
    cols = np.arange(NP)
    cst = np.zeros((128, 6), np.float32)
    cst[0:64, 0] = 1.0
    cst[64:128, 1] = 1.0
    cst[0:64, 5] = 1.0
    for k in range(NCORES):
        sl_chunks = chunks_frame[k * NCH:(k + 1) * NCH]
        cpts = chunks_pts[k * NCH:(k + 1) * NCH]         # (NCH, NP)
        cval = valid[k * NCH:(k + 1) * NCH]
        cx = xs[k * NCH:(k + 1) * NCH]                   # (NCH, NP, 4)

        sp = slabs16[sl_chunks]                          # (NCH, 3, 256, 192)
        sp = sp.reshape(NCH, 3, 2, 128, 192).transpose(3, 0, 1, 2, 4)
        sl_arr = np.ascontiguousarray(sp).reshape(128, NCH * 6 * 192)

        m_arr = np.zeros((NCH, 3, 257, NP), np.float32)
        for j in range(3):
            ix = 255.0 * cx[:, :, j]
            x0 = np.floor(ix).astype(np.int64)
            wx = (ix - x0).astype(np.float32)
            w0 = np.where(cval, 1.0 - wx, 0.0).astype(np.float32)
            w1 = np.where(cval, wx, 0.0).astype(np.float32)
            ci = np.broadcast_to(np.arange(NCH)[:, None], x0.shape)
            cj = np.broadcast_to(cols[None, :], x0.shape)
            m_arr[ci, j, x0, cj] = w0
            m_arr[ci, j, x0 + 1, cj] = w1
        m16 = m_arr[:, :, :256, :].astype(bf16)          # (NCH, 3, 256, NP)
        m16 = m16.reshape(NCH, 3, 2, 128, NP).transpose(3, 0, 1, 2, 4)
        m_arr2 = np.ascontiguousarray(m16).reshape(128, NCH * 6 * NP)

        # xyz in device order: [i, s, (cc*BLK + b)*3 + d]
        cxyz = cx[:, :, 0:3].reshape(NSUP, STACK, BLK, 128, 3)
        cxv = cval.reshape(NSUP, STACK, BLK, 128)
        xyz_pack = np.where(cxv[..., None], cxyz, 0.0)
        xyz_pack = xyz_pack.transpose(3, 0, 1, 2, 4)     # (128, NSUP, STACK, BLK, 3)
        xyz_arr = np.ascontiguousarray(xyz_pack).reshape(128, NSUP * SM * 3)

        in_maps.append({
            "m_in": m_arr2, "sl_in": sl_arr,
            "xyz_in": xyz_arr.astype(np.float32), "cst_in": cst,
        })
        pts_r = cpts.reshape(NSUP, STACK, BLK, 128).transpose(3, 0, 1, 2)
        scatter.append(np.ascontiguousarray(pts_r))      # (128, NSUP, STACK, BLK)
    return in_maps, scatter


_NC_CACHE = {}


def _get_nc():
    if "nc" not in _NC_CACHE:
        _NC_CACHE["nc"] = build_program()
    return _NC_CACHE["nc"]


def kernel(x, feat0, feat1, feat2):
    nc = _get_nc()
    in_maps, scatter = pack_inputs(x, feat0, feat1, feat2)
    res = run_bass_kernel_spmd(nc, in_maps, core_ids=list(range(NCORES)))
    out = np.empty((P, 63), np.float32)
    for k in range(NCORES):
        o = res.results[k]["out"].reshape(128, NSUP, SM, 63)
        pts = scatter[k].reshape(128, NSUP, SM)
        m = pts >= 0
        out[pts[m]] = o[m].astype(np.float32)
    return out


# revision 6
# speedup vs baseline: 3.5033x; 1.2705x over previous
"""DNeRF-TensoRF sampler kernel for Trainium2 (8 NeuronCores).

The reference bilinearly samples 9 feature planes (3 feats x 3 xyz-components)
at (x_j, t) per point; t is an exact integer frame, so sampling reduces to a
1-D lerp along x within frame t's 256-column slab.

Strategy (no per-point DMA gathers -- SWDGE descriptor generation is ~8.5ns
per descriptor and would cost ~1.7ms/core for 196k row fetches):
  - Host sorts points by frame and packs them into frame-aligned chunks of
    NP=512 points (138 chunks/core incl. dummy padding).
  - For each chunk the host ships (a) the frame's 3 slabs (256 rows x 192
    channels, bf16) and (b) three dense "hat" matrices M_j (256 x NP, bf16)
    holding the two lerp taps (1-wx, wx) per column.
  - The device computes s_j = Slab_j^T @ M_j on the tensor engine (the whole
    gather+lerp as matmuls), multiplies the three samples on DVE, and reduces
    the 64-channel blocks with a second matmul whose stationary operand is the
    product tile itself -- which lands the per-point deltas point-major in
    PSUM, ready for the positional encoding (angle-doubling recursion with
    ADD_RANGE_WRAP + ACT Sin).
  - Outputs (fp16) are unpermuted/cast on host.
"""
import sys

sys.path.insert(0, "/opt/trn_rl_repo")

from contextlib import ExitStack

import numpy as np
import ml_dtypes

import concourse.bacc as bacc
import concourse.bass as bass
import concourse.mybir as mybir
import concourse.tile as tile
from concourse.bass_utils import run_bass_kernel_spmd
from concourse.dve_ops import ADD_RANGE_WRAP

NUM_FRAMES = 100
RESO = 256
CHAN = 64
FREQ = 10
P = 524288
NCORES = 8

NP = 512                    # points per chunk (one frame per chunk)
NCH = 138                   # chunks per core
GCH = NCORES * NCH          # 1104 global chunks
STACK = 23                  # chunks per output super (138 = 6*23)
NSUP = NCH // STACK         # 6 supers
BLK = NP // 128             # 4 x 128-point blocks per chunk
SM = STACK * BLK            # 92 points per partition per super

bf16 = ml_dtypes.bfloat16
TWO_PI = float(2 * np.pi)
PI = float(np.pi)
HALF_PI = float(np.pi / 2)


def build_program():
    f32 = mybir.dt.float32
    f16 = mybir.dt.float16
    b16 = mybir.dt.bfloat16
    A = mybir.AluOpType

    nc = bacc.Bacc("TRN2", target_bir_lowering=False, debug=False)

    m_in = nc.dram_tensor("m_in", [128, NCH * 6 * NP], b16, kind="ExternalInput")
    sl_in = nc.dram_tensor("sl_in", [128, NCH * 6 * 192], b16, kind="ExternalInput")
    xyz_in = nc.dram_tensor("xyz_in", [128, NSUP * SM * 3], f32, kind="ExternalInput")
    cst_in = nc.dram_tensor("cst_in", [128, 6], f32, kind="ExternalInput")
    out = nc.dram_tensor("out", [128, NSUP * SM * 63], f16, kind="ExternalOutput")

    m_v = m_in.ap().rearrange("p (c a n) -> p c a n", a=6, n=NP)
    sl_v = sl_in.ap().rearrange("p (c a n) -> p c a n", a=6, n=192)
    out_v = out.ap().rearrange("p (s k) -> p s k", k=SM * 63)

    with tile.TileContext(nc) as tc, ExitStack() as ctx:
        cpool = ctx.enter_context(tc.tile_pool(name="const", bufs=1))
        cst = cpool.tile([128, 6], f32)
        nc.sync.dma_start(cst[:], cst_in.ap()[:])
        selw = cpool.tile([128, 6], b16)
        nc.vector.tensor_copy(selw[:], cst[:])            # cast f32 -> bf16
        xyzt = cpool.tile([128, NSUP * SM * 3], f32)
        nc.sync.dma_start(xyzt[:], xyz_in.ap()[:])
        xyz_t = xyzt[:].rearrange("p (s m d) -> p s m d", m=SM, d=3)

        mpool = ctx.enter_context(tc.tile_pool(name="m", bufs=3))
        slpool = ctx.enter_context(tc.tile_pool(name="sl", bufs=3))
        sbpool = ctx.enter_context(tc.tile_pool(name="sb", bufs=2))
        xpool = ctx.enter_context(tc.tile_pool(name="x", bufs=2))
        opool = ctx.enter_context(tc.tile_pool(name="o", bufs=2))
        pp = ctx.enter_context(tc.psum_pool(name="ps", bufs=2))

        for s in range(NSUP):
            dpm = pp.tile([128, STACK, BLK, 3], f32, tag="dpm")
            for cc in range(STACK):
                c = s * STACK + cc
                mt = mpool.tile([128, 6, NP], b16, tag="mt")
                nc.sync.dma_start(mt[:], m_v[:, c])
                sl = slpool.tile([128, 6, 192], b16, tag="sl")
                nc.sync.dma_start(sl[:], sl_v[:, c])

                sps = []
                for j in range(3):
                    sA = pp.tile([128, NP], f32, tag="sA", bufs=3)
                    sB = pp.tile([64, NP], f32, tag="sB", bufs=3)
                    nc.tensor.matmul(sA[:], sl[:, 2 * j, 0:128], mt[:, 2 * j, :],
                                     start=True, stop=False)
                    nc.tensor.matmul(sA[:], sl[:, 2 * j + 1, 0:128],
                                     mt[:, 2 * j + 1, :], start=False, stop=True)
                    nc.tensor.matmul(sB[:], sl[:, 2 * j, 128:192],
                                     mt[:, 2 * j, :], start=True, stop=False)
                    nc.tensor.matmul(sB[:], sl[:, 2 * j + 1, 128:192],
                                     mt[:, 2 * j + 1, :], start=False, stop=True)
                    sps.append((sA, sB))

                s1A = sbpool.tile([128, NP], b16, tag="s1A")
                nc.scalar.copy(s1A[:], sps[1][0][:])
                s1B = sbpool.tile([64, NP], b16, tag="s1B")
                nc.scalar.copy(s1B[:], sps[1][1][:])

                p01A = sbpool.tile([128, NP], b16, tag="p01A")
                nc.vector.tensor_tensor(p01A[:], sps[0][0][:], s1A[:], A.mult)
                p01B = sbpool.tile([64, NP], b16, tag="p01B")
                nc.vector.tensor_tensor(p01B[:], sps[0][1][:], s1B[:], A.mult)

                prA = sbpool.tile([128, NP], b16, tag="prA")
                nc.vector.tensor_tensor(prA[:], p01A[:], sps[2][0][:], A.mult)
                prB = sbpool.tile([64, NP], b16, tag="prB")
                nc.vector.tensor_tensor(prB[:], p01B[:], sps[2][1][:], A.mult)

                # channel reduce with point-major output:
                # dpm[i, cc, b, f] = sum_c pr[c, b*128+i] * selw[c, f]
                for b in range(BLK):
                    nc.tensor.matmul(dpm[:, cc, b, :],
                                     prA[:, b * 128:(b + 1) * 128],
                                     selw[:, 0:3], start=True, stop=False)
                    nc.tensor.matmul(dpm[:, cc, b, :],
                                     prB[:, b * 128:(b + 1) * 128],
                                     selw[0:64, 3:6], start=False, stop=True)

            px = xpool.tile([128, SM, 3], f32, tag="px")
            nc.vector.tensor_tensor(
                px[:], dpm[:].rearrange("p st b d -> p (st b) d"), xyz_t[:, s],
                A.add)

            ot = opool.tile([128, SM, 63], f16, tag="ot")
            nc.scalar.copy(ot[:, :, 0:3], px[:])

            th = px
            for k in range(FREQ):
                nc.scalar.activation(
                    ot[:, :, 3 + 6 * k:6 + 6 * k], th[:],
                    mybir.ActivationFunctionType.Sin)
                cw = xpool.tile([128, SM, 3], f32, tag="cw")
                nc.vector._custom_dve(ADD_RANGE_WRAP, out=cw[:], in0=th[:],
                                      s0=HALF_PI, s1=PI, imm2=TWO_PI)
                nc.scalar.activation(
                    ot[:, :, 6 + 6 * k:9 + 6 * k], cw[:],
                    mybir.ActivationFunctionType.Sin)
                if k < FREQ - 1:
                    th2 = xpool.tile([128, SM, 3], f32, tag="th2")
                    nc.vector.tensor_scalar(th2[:], th[:], 2.0, None, A.mult)
                    thn = xpool.tile([128, SM, 3], f32, tag="thn")
                    nc.vector._custom_dve(ADD_RANGE_WRAP, out=thn[:], in0=th2[:],
                                          s0=0.0, s1=PI, imm2=TWO_PI)
                    th = thn

            nc.sync.dma_start(out_v[:, s], ot[:].rearrange("p m k -> p (m k)"))

    nc.compile()
    return nc


def pack_inputs(x, feat0, feat1, feat2):
    """Sort by frame, chunk, and build per-core input maps + the scatter map
    for unpermuting device outputs."""
    x = np.asarray(x, np.float32)
    t_all = x[:, 3].astype(np.int32)

    order = np.argsort(t_all, kind="stable")
    t_sorted = t_all[order]
    counts = np.bincount(t_sorted, minlength=NUM_FRAMES)
    chunks_frame = []
    chunks_pts = []
    pos = 0
    for f in range(NUM_FRAMES):
        n = counts[f]
        fpts = order[pos:pos + n]
        pos += n
        for a in range(0, n, NP):
            seg = fpts[a:a + NP]
            if len(seg) < NP:
                seg = np.concatenate(
                    [seg, np.full(NP - len(seg), -1, np.int64)])
            chunks_frame.append(f)
            chunks_pts.append(seg)
    assert len(chunks_frame) <= GCH, f"{len(chunks_frame)} chunks > {GCH}"
    while len(chunks_frame) < GCH:
        chunks_frame.append(0)
        chunks_pts.append(np.full(NP, -1, np.int64))
    chunks_frame = np.asarray(chunks_frame, np.int32)
    chunks_pts = np.stack(chunks_pts)                    # (GCH, NP)

    # per-frame slabs: slab[f][j] = (256 rows, 192 ch), f-major channels
    slabs = np.empty((NUM_FRAMES, 3, 256, 192), np.float32)
    for j in range(3):
        planes = np.stack([np.asarray(feat0, np.float32)[j],
                           np.asarray(feat1, np.float32)[j],
                           np.asarray(feat2, np.float32)[j]], axis=0)
        slabs[:, j] = planes.transpose(2, 3, 0, 1).reshape(
            NUM_FRAMES, 256, 192)
    slabs16 = slabs.astype(bf16)

    valid = chunks_pts >= 0                              # (GCH, NP)
    safe_pts = np.where(valid, chunks_pts, 0)
    xs = x[safe_pts]                                     # (GCH, NP, 4)
    in_maps = []
    scatter = []
    cols = np.arange(NP)
    cst = np.zeros((128, 6), np.float32)
    cst[0:64, 0] = 1.0
    cst[64:128, 1] = 1.0
    cst[0:64, 5] = 1.0
    for k in range(NCORES):
        sl_chunks = chunks_frame[k * NCH:(k + 1) * NCH]
        cpts = chunks_pts[k * NCH:(k + 1) * NCH]         # (NCH, NP)
        cval = valid[k * NCH:(k + 1) * NCH]
        cx = xs[k * NCH:(k + 1) * NCH]                   # (NCH, NP, 4)

        sp = slabs16[sl_chunks]                          # (NCH, 3, 256, 192)
        sp = sp.reshape(NCH, 3, 2, 128, 192).transpose(3, 0, 1, 2, 4)
        sl_arr = np.ascontiguousarray(sp).reshape(128, NCH * 6 * 192)

        m_arr = np.zeros((NCH, 3, 257, NP), np.float32)
        for j in range(3):
            ix = 255.0 * cx[:, :, j]
            x0 = np.floor(ix).astype(np.int64)
            wx = (ix - x0).astype(np.float32)
            w0 = np.where(cval, 1.0 - wx, 0.0).astype(np.float32)
            w1 = np.where(cval, wx, 0.0).astype(np.float32)
            ci = np.broadcast_to(np.arange(NCH)[:, None], x0.shape)
            cj = np.broadcast_to(cols[None, :], x0.shape)
            m_arr[ci, j, x0, cj] = w0
            m_arr[ci, j, x0 + 1, cj] = w1
        m16 = m_arr[:, :, :256, :].astype(bf16)          # (NCH, 3, 256, NP)
        m16 = m16.reshape(NCH, 3, 2, 128, NP).transpose(3, 0, 1, 2, 4)
        m_arr2 = np.ascontiguousarray(m16).reshape(128, NCH * 6 * NP)

        # xyz in device order: [i, s, (cc*BLK + b)*3 + d]
        cxyz = cx[:, :, 0:3].reshape(NSUP, STACK, BLK, 128, 3)
        cxv = cval.reshape(NSUP, STACK, BLK, 128)
        xyz_pack = np.where(cxv[..., None], cxyz, 0.0)
        xyz_pack = xyz_pack.transpose(3, 0, 1, 2, 4)     # (128, NSUP, STACK, BLK, 3)
        xyz_arr = np.ascontiguousarray(xyz_pack).reshape(128, NSUP * SM * 3)

        in_maps.append({
            "m_in": m_arr2, "sl_in": sl_arr,
            "xyz_in": xyz_arr.astype(np.float32), "cst_in": cst,
        })
        pts_r = cpts.reshape(NSUP, STACK, BLK, 128).transpose(3, 0, 1, 2)
        scatter.append(np.ascontiguousarray(pts_r))      # (128, NSUP, STACK, BLK)
    return in_maps, scatter


_NC_CACHE = {}


def _get_nc():
    if "nc" not in _NC_CACHE:
        _NC_CACHE["nc"] = build_program()
    return _NC_CACHE["nc"]


def kernel(x, feat0, feat1, feat2):
    nc = _get_nc()
    in_maps, scatter = pack_inputs(x, feat0, feat1, feat2)
    res = run_bass_kernel_spmd(nc, in_maps, core_ids=list(range(NCORES)))
    out = np.empty((P, 63), np.float32)
    for k in range(NCORES):
        o = res.results[k]["out"].reshape(128, NSUP, SM, 63)
        pts = scatter[k].reshape(128, NSUP, SM)
        m = pts >= 0
        out[pts[m]] = o[m].astype(np.float32)
    return out
